# revision 21
# baseline (speedup 1.0000x reference)
"""Segment-mean (word-pooling) kernel for Trainium2, 8 NeuronCores.

Problem: hidden_states [16, 4096, 768] f32, word_ids [16, 4096] i32
(non-decreasing per row, -1 = special token). Output [16, 2048, 768] f32:
mean of each word's subword embeddings; words with no tokens -> 0.

Active design (V2, word-aligned chunking; the older V1 banded-window
formulation is kept below for reference):

Pure data parallelism, 2 samples per core. The valid-token stream of
each sample (positions 1..4094 here; -1 only marks CLS/SEP) is chunked
host-side into pieces of <=128 tokens covering <=64 WHOLE words — no
word ever straddles a chunk — so each chunk needs exactly ONE one-hot
matmul (K=128 tokens, M=64 words, N=768) into its own psum half:
  ps[w, h] = sum_tok onehot[tok, w] * x[tok, h]
Two chunks pair into one [128, 768] psum tile via PE column tiling
(tile_position (0,0)/(0,64)); the per-word 1/count (host-computed,
carries the fp8 descale and f8 output scale) is applied during the
PSUM->SBUF eviction, alternated between the scalar and vector engines.
All data-dependence (chunk boundaries) lives in the host-packed x/meta
tensors; the device program is completely data-independent, so the SPMD
program is trivially identical across cores (samples are paired to
cores largest+smallest to balance chunk counts).

Memory-path choices, all validated on HW:
  - x is fp8-e3m4 (host pre-scaled by 2, descale folded into 1/count);
    one-hots stay exact in fp8 (pure 0/1). Output is ALSO fp8-e3m4
    (host pre-scaled by 2): end-to-end rel err 1.525e-2 vs the 2e-2
    gate. Per-core HBM traffic: 6.68 MB in + 3.34 MB out ~= the 28 us
    HBM-per-core roofline (358 GB/s), which measurement shows is the
    binding resource - the body runs within ~4 us of it.
  - input ships chunk-major [nchunks, 128, 768] so every DMA line is
    one contiguous multi-KB chunk per partition; input DMAs ride the
    sync HWDGE ring, output the scalar ring. Splitting input DMAs
    across both rings measured WORSE (the scalar engine's strict-FIFO
    queue delays DMA issue behind evictions).
  - measured on this stack: a hardware For_i iteration carries a fixed
    ~46 us reset/barrier cost even with an EMPTY body, so steady-state
    timing (test.py) unrolls R_UNROLL kernel bodies per loop iteration
    and reports the marginal per-body slope.
"""

import numpy as np

B, S, H = 16, 4096, 768
NUM_WORDS = S // 2  # 2048
N_CORES = 8
SPC = B // N_CORES  # samples per core = 2
P = 128
KT = S // P  # 32 k-tiles per sample
NW = NUM_WORDS // P  # 16 output windows per sample
NSPLITS = ((0, 512), (512, 768))  # matmul free-dim splits of H


def _plan(word_ids: np.ndarray):
    """Per-slot union plan. Samples are assigned to slots to minimize the
    total union window-span (= matmul pairs): for each slot and k-tile t
    the span [minwin, maxwin] over that slot's 8 samples; per window j
    the sorted member k-tiles. Returns (plans, perm) where plans is
    (spans, members) per slot and perm[c][s] = sample of core c, slot s."""
    word_ids = np.minimum(word_ids, NUM_WORDS - 1)
    minw = np.full((B, KT), NW, np.int64)
    maxw = np.full((B, KT), -1, np.int64)
    for b in range(B):
        row = word_ids[b]
        for t in range(KT):
            w = row[t * P : (t + 1) * P]
            w = w[w >= 0]
            if w.size:
                minw[b, t] = w.min() // P
                maxw[b, t] = w.max() // P
    if SPC == 2:
        from itertools import combinations

        allb = set(range(B))
        best = None
        for g0r in combinations(range(1, B), N_CORES - 1):
            g0 = [0, *g0r]
            g1 = sorted(allb - set(g0))
            cost = 0
            for g in (g0, g1):
                cost += int(
                    np.sum(maxw[g].max(0) - np.minimum(minw[g].min(0), maxw[g].max(0)) + 1)
                )
            if best is None or cost < best[0]:
                best = (cost, g0, g1)
        groups = [best[1], best[2]]
    else:
        groups = [list(range(s, B, SPC)) for s in range(SPC)]
    perm = np.array(groups, dtype=np.int64).T  # [N_CORES, SPC]
    plans = []
    for g in groups:
        mn = np.minimum(minw[g].min(0), maxw[g].max(0))
        mx = maxw[g].max(0)
        members = {j: [] for j in range(NW)}
        spans = []
        for t in range(KT):
            spans.append((int(mn[t]), int(mx[t])))
            for j in range(int(mn[t]), int(mx[t]) + 1):
                members[j].append(t)
        plans.append((spans, members))
    return plans, perm


def _liveness(plans, in_group):
    """Max number of simultaneously-live x DMA groups / onehot tiles over
    the per-window emission order, across slots. A pool needs at least
    this many bufs or slot reuse can deadlock the DMA ring."""
    max_live_g, max_live_oh = 0, 0
    for spans, members in plans:
        first_g, last_g, first_oh, last_oh = {}, {}, {}, {}
        for j in range(NW):
            for t in members[j]:
                g = t // in_group
                first_g.setdefault(g, j)
                last_g[g] = j
                first_oh.setdefault(t, j)
                last_oh[t] = j
        for j in range(NW):
            live_g = sum(1 for g in first_g if first_g[g] <= j <= last_g[g])
            live_oh = sum(1 for t in first_oh if first_oh[t] <= j <= last_oh[t])
            max_live_g = max(max_live_g, live_g)
            max_live_oh = max(max_live_oh, live_oh)
    return max_live_g, max_live_oh


def _recip_counts(word_ids: np.ndarray) -> np.ndarray:
    """Per-token 1/count(word) as f32; 0 for special (-1) tokens."""
    r = np.zeros((B, S), np.float32)
    for b in range(B):
        wid = word_ids[b]
        valid = wid >= 0
        counts = np.bincount(wid[valid], minlength=NUM_WORDS)
        r[b, valid] = (1.0 / counts[wid[valid]]).astype(np.float32)
    return r


def _build(
    plans,
    reps=1,
    dyn_reps=1,
    do_mm=True,
    do_out=True,
    do_in=True,
    x_bufs=8,
    oh_bufs=8,
    ev_bufs=4,
    ps_bufs=3,
    in_group=4,
    out_group=2,
    in_alt=False,
    out_engine="scalar",
    in_dtype="f16",
    out_dtype="f16",
    n_splits=None,
    ev_engine="scalar",
    in_layout="rowmajor",
    out_layout="rowmajor",
    rcp_mode="onehot",
    split_psum=False,
    do_ev=True,
    oh_const=False,
    interleave=False,
):
    """Build + compile the SPMD Bass program. reps>1 unrolls the whole
    body; dyn_reps>1 wraps it in a hardware For loop — both only used
    for amortized wall-clock timing. do_* flags ablate kernel stages
    for benchmarking (outputs are wrong when any is False)."""
    from contextlib import nullcontext
    import concourse.bacc as bacc
    import concourse.tile as tile
    from concourse import mybir

    nc = bacc.Bacc(
        "TRN2",
        target_bir_lowering=False,
        debug=False,
        enable_asserts=False,
        num_devices=N_CORES,
    )
    f32 = mybir.dt.float32
    fin = {"f16": mybir.dt.float16, "f8e3": mybir.dt.float8e3, "f32": f32}[in_dtype]
    fout = mybir.dt.float16 if out_dtype == "f16" else f32
    assert rcp_mode in ("onehot", "evict")
    if in_dtype == "f8e3":
        assert rcp_mode == "evict", "fp8 one-hot cannot carry 1/count exactly"
    fused = in_layout == "pmajor_fused"
    if in_layout == "pmajor":
        x = nc.dram_tensor(
            "x", [SPC * KT // in_group, P, in_group * H], fin, kind="ExternalInput"
        ).ap()
    elif fused:
        # both slots' k-tile group g in one DMA: halves dma_start count
        x = nc.dram_tensor(
            "x", [KT // in_group, P, SPC * in_group * H], fin, kind="ExternalInput"
        ).ap()
    else:
        x = nc.dram_tensor("x", [SPC * S, H], fin, kind="ExternalInput").ap()
    # onehot mode: per-token 1/count [.., KT]; evict mode: per-word [.., NW];
    # word ids and 1/count ship fused in one tensor -> one prologue DMA
    rcp_w = KT if rcp_mode == "onehot" else NW
    meta = nc.dram_tensor(
        "meta", [P, SPC * (KT + rcp_w)], f32, kind="ExternalInput"
    ).ap()
    if out_layout == "pmajor":
        y = nc.dram_tensor(
            "y", [SPC * (NW // out_group), P, out_group * H], fout,
            kind="ExternalOutput",
        ).ap()
    else:
        y = nc.dram_tensor(
            "y", [SPC * NUM_WORDS, H], fout, kind="ExternalOutput"
        ).ap()

    IOTA_W = NUM_WORDS + 2 * P  # ramp long enough for any window pair
    max_span = max(
        (jhi - jlo + 1) for spans, _ in plans for (jlo, jhi) in spans
    )

    # Size pools from plan liveness; degenerate plans (heavily overlapping
    # window k-ranges) fall back to a bounded reload mode, otherwise pool
    # slot reuse can deadlock the DMA ring.
    in_b = {"f16": 2, "f8e3": 1, "f32": 4}[in_dtype]
    live_g, live_oh = _liveness(plans, in_group)
    if fused:
        # every fused group tile stays live from slot 0 use to slot 1 use
        need_x, need_oh = KT // in_group + 2, live_oh + 3
        x_bytes = need_x * SPC * in_group * H * in_b
    else:
        need_x, need_oh = live_g + 3, live_oh + 3
        x_bytes = need_x * in_group * H * in_b
    oh_bytes = need_oh * max_span * P * in_b
    safe = x_bytes + oh_bytes > 150 * 1024
    if not safe:
        x_bufs = max(x_bufs, need_x)
        oh_bufs = max(oh_bufs, need_oh)
    assert not (fused and safe), "fused input layout incompatible with safe mode"

    with tile.TileContext(nc) as tc:
        with (
            tc.tile_pool(name="const", bufs=1) as const_pool,
            tc.tile_pool(name="xin", bufs=x_bufs) as x_pool,
            tc.tile_pool(name="oh", bufs=oh_bufs) as oh_pool,
            tc.tile_pool(name="ev", bufs=ev_bufs) as ev_pool,
            tc.tile_pool(
                name="psum", bufs=(4 if split_psum else ps_bufs), space="PSUM"
            ) as psum_pool,
            tc.tile_pool(name="psumb", bufs=4, space="PSUM") as psumb_pool,
        ):
            # f32 iota is exact for our word-index range (< 2^24): build the
            # ramp directly, no int staging tile + DVE copy in the prologue
            iota_f = const_pool.tile([P, IOTA_W], f32)
            nc.gpsimd.iota(
                iota_f[:], pattern=[[1, IOTA_W]], base=0, channel_multiplier=0,
                allow_small_or_imprecise_dtypes=True,
            )
            oh_c = None
            if oh_const:  # ablation: one fixed oh tile, no per-tile DVE builds
                oh_c = const_pool.tile([P, P], fin, name="oh_const")
                nc.vector.memset(oh_c[:], 0.0)

            IG, OG = in_group, out_group
            out_eng = nc.sync if out_engine == "sync" else nc.scalar
            ev_eng = nc.vector if ev_engine == "vector" else nc.scalar
            splits = NSPLITS if n_splits is None else n_splits

            # wid/rcp are loop constants: ONE fused DMA, outside any reps loop
            SW = KT + rcp_w
            meta_t = const_pool.tile([P, SPC * SW], f32, name="meta")
            nc.scalar.dma_start(out=meta_t[:], in_=meta[:, :])
            wid_ts = [meta_t[:, s * SW : s * SW + KT] for s in range(SPC)]
            rcp_ts = [meta_t[:, s * SW + KT : (s + 1) * SW] for s in range(SPC)]

            def emit(rep):
                fused_tiles = {}

                def slot_body(slot):
                    spans, members = plans[slot]
                    wid_t = wid_ts[slot]
                    rcp_t = rcp_ts[slot]

                    def build_oh(out_ap, iota_ap, t):
                        if rcp_mode == "onehot":
                            nc.vector.tensor_scalar(
                                out=out_ap,
                                in0=iota_ap,
                                scalar1=wid_t[:, t : t + 1],
                                scalar2=rcp_t[:, t : t + 1],
                                op0=mybir.AluOpType.is_equal,
                                op1=mybir.AluOpType.mult,
                            )
                        else:
                            nc.vector.tensor_scalar(
                                out=out_ap,
                                in0=iota_ap,
                                scalar1=wid_t[:, t : t + 1],
                                scalar2=None,
                                op0=mybir.AluOpType.is_equal,
                            )

                    def evict(out_ap, ps_ap, j, eng):
                        if rcp_mode == "onehot":
                            if eng == "vector":
                                nc.vector.tensor_copy(out=out_ap, in_=ps_ap)
                            else:
                                nc.scalar.copy(out=out_ap, in_=ps_ap)
                        else:
                            if eng == "vector":
                                nc.vector.tensor_scalar(
                                    out=out_ap,
                                    in0=ps_ap,
                                    scalar1=rcp_t[:, j : j + 1],
                                    scalar2=None,
                                    op0=mybir.AluOpType.mult,
                                )
                            else:
                                nc.scalar.mul(
                                    out=out_ap, in_=ps_ap, mul=rcp_t[:, j : j + 1]
                                )

                    xg_tiles = {}
                    oh_tiles = {}

                    def dma_eng(g):
                        if in_alt == "gpsimd":
                            return nc.gpsimd if g % 2 == 1 else nc.sync
                        if in_alt == "3way":
                            return (nc.sync, nc.scalar, nc.gpsimd)[g % 3]
                        if in_alt:
                            return nc.scalar if g % 2 == 1 else nc.sync
                        return nc.sync

                    def get_x(t):
                        g, a = divmod(t, IG)
                        if fused:
                            if g not in fused_tiles:
                                xt = x_pool.tile(
                                    [P, SPC, IG, H], fin,
                                    name=f"xt_{rep}_{g}", tag="xt",
                                )
                                if do_in:
                                    src = x[g, :, :].rearrange(
                                        "p (s a h) -> p s a h", s=SPC, a=IG
                                    )
                                    dma_eng(g).dma_start(out=xt[:], in_=src)
                                fused_tiles[g] = xt
                            return fused_tiles[g][:, slot, a, :]
                        if g not in xg_tiles:
                            xt = x_pool.tile(
                                [P, IG, H], fin, name=f"xt_{rep}_{slot}_{g}", tag="xt"
                            )
                            if do_in:
                                if in_layout == "pmajor":
                                    src = x[slot * (KT // IG) + g, :, :].rearrange(
                                        "p (a h) -> p a h", a=IG
                                    )
                                else:
                                    r0 = slot * S + g * IG * P
                                    src = x[r0 : r0 + IG * P, :].rearrange(
                                        "(a p) h -> p a h", p=P
                                    )
                                dma_eng(g).dma_start(out=xt[:], in_=src)
                            xg_tiles[g] = xt
                        return xg_tiles[g][:, t % IG, :]

                    def get_oh(t):
                        if oh_const:
                            return oh_c
                        if t not in oh_tiles:
                            jlo, jhi = spans[t]
                            wspan = (jhi - jlo + 1) * P
                            oh = oh_pool.tile(
                                [P, max_span * P],
                                fin,
                                name=f"oh_{rep}_{slot}_{t}",
                                tag="oh",
                            )
                            build_oh(
                                oh[:, :wspan],
                                iota_f[:, jlo * P : jlo * P + wspan],
                                t,
                            )
                            oh_tiles[t] = oh
                        return oh_tiles[t]

                    og_tile = [None]

                    for j in range(NW):
                        if j % OG == 0:
                            og_tile[0] = ev_pool.tile(
                                [P, OG, H], fout, name=f"out_{rep}_{slot}_{j}", tag="out"
                            )
                        out_sb = og_tile[0][:, j % OG, :]
                        ks = members[j]
                        if not do_mm:
                            for t in ks:
                                get_x(t)
                                get_oh(t)
                        if not ks:
                            nc.vector.memset(out_sb, 0.0)
                        elif not do_mm:
                            if do_out:
                                nc.gpsimd.memset(out_sb, 0.0)
                        else:
                            if split_psum:
                                pss = [
                                    (psum_pool if si == 0 else psumb_pool).tile(
                                        [P, hi - lo], f32,
                                        name=f"ps{si}_{rep}_{slot}_{j}",
                                        tag=f"ps{si}",
                                    )
                                    for si, (lo, hi) in enumerate(splits)
                                ]
                            else:
                                ps = psum_pool.tile(
                                    [P, H], f32, name=f"ps_{rep}_{slot}_{j}", tag="ps"
                                )
                            for ki, t in enumerate(ks):
                                if safe:
                                    xs = x_pool.tile(
                                        [P, 1, H], fin,
                                        name=f"xs_{rep}_{slot}_{j}_{t}", tag="xt",
                                    )
                                    r0 = slot * S + t * P
                                    nc.sync.dma_start(
                                        out=xs[:], in_=x[r0 : r0 + P, :].rearrange(
                                            "(a p) h -> p a h", p=P
                                        )
                                    )
                                    xt = xs[:, 0, :]
                                    oh = oh_pool.tile(
                                        [P, P], fin,
                                        name=f"ohs_{rep}_{slot}_{j}_{t}", tag="oh",
                                    )
                                    build_oh(
                                        oh[:, :],
                                        iota_f[:, j * P : (j + 1) * P],
                                        t,
                                    )
                                    off = 0
                                else:
                                    xt = get_x(t)
                                    oh = get_oh(t)
                                    off = 0 if oh_const else (j - spans[t][0]) * P
                                for si, (lo, hi) in enumerate(splits):
                                    nc.tensor.matmul(
                                        out=(
                                            pss[si][:, :]
                                            if split_psum
                                            else ps[:, lo:hi]
                                        ),
                                        lhsT=oh[:, off : off + P],
                                        rhs=xt[:, lo:hi],
                                        start=(ki == 0),
                                        stop=(ki == len(ks) - 1),
                                    )
                            if not do_ev:
                                pass
                            elif split_psum:
                                for si, (lo, hi) in enumerate(splits):
                                    eng = (
                                        ("scalar" if si == 0 else "vector")
                                        if ev_engine == "alt"
                                        else ev_engine
                                    )
                                    evict(out_sb[:, lo:hi], pss[si][:, :], j, eng)
                            elif ev_engine == "alt":
                                evict(out_sb, ps, j, "scalar" if j % 2 == 0 else "vector")
                            else:
                                evict(out_sb, ps, j, ev_engine)
                        if do_out and j % OG == OG - 1:
                            if out_layout == "pmajor":
                                dst = y[
                                    slot * (NW // OG) + j // OG, :, :
                                ].rearrange("p (a h) -> p a h", a=OG)
                            else:
                                r0 = slot * NUM_WORDS + (j - OG + 1) * P
                                dst = y[r0 : r0 + OG * P, :].rearrange(
                                    "(a p) h -> p a h", p=P
                                )
                            if out_engine == "alt":
                                oe = nc.scalar if (j // OG) % 2 == 0 else nc.sync
                            elif out_engine == "swdge_alt":
                                oe = nc.scalar if (j // OG) % 2 == 0 else nc.gpsimd
                            else:
                                oe = out_eng
                            oe.dma_start(out=dst, in_=og_tile[0][:])
                        yield

                gens = [slot_body(s) for s in range(SPC)]
                if interleave:
                    # alternate windows across slots: two independent dep
                    # chains keep engines fed when one chain stalls
                    done = [False] * SPC
                    while not all(done):
                        for i, g in enumerate(gens):
                            if not done[i]:
                                try:
                                    next(g)
                                except StopIteration:
                                    done[i] = True
                else:
                    for g in gens:
                        for _ in g:
                            pass

            loop_cm = (
                tc.For_i(0, dyn_reps, 1) if dyn_reps > 1 else nullcontext()
            )
            with loop_cm:
                for rep in range(reps):
                    emit(rep)

    nc.compile()
    return nc


FP8_SCALE = 2.0  # host-side pre-scale for f8e3 inputs; undone via rcp


# ---------------------------------------------------------------------------
# V2: word-aligned chunked formulation.
#
# Chunk the (valid) token stream of each sample into pieces of <=TC=128
# tokens covering <=WC=64 whole words (no word straddles a chunk), so each
# chunk needs exactly ONE one-hot matmul (M=64 words, K=128 tokens, N=768)
# into its own psum half. Two chunks pair into one [128, 768] psum tile via
# PE column tiling (tile_position (0,0)/(0,64)), which the PE can run
# concurrently. Output is written chunk-major and re-assembled on host.
# All data-dependence (chunk boundaries) lives in host-packed x/meta; the
# device program is data-independent.
# ---------------------------------------------------------------------------

TC = 128  # tokens per chunk (PE contraction dim)
WC = 64  # words per chunk (half a psum tile)


def _plan2(word_ids: np.ndarray, pad_mult=8):
    """Chunk each sample; assign samples to cores pairing large with small
    chunk counts. Returns (percore, nct) where percore[c] is a list of
    chunk dicts (sample, t0, ntok, w0, nw) padded to common length nct."""
    wid = np.minimum(np.asarray(word_ids, dtype=np.int64), NUM_WORDS - 1)
    chunks_by_sample = []
    for b in range(B):
        row = wid[b]
        valid = row >= 0
        w = row[valid]
        t0s = np.nonzero(valid)[0]
        # valid tokens must be one contiguous run for slice-based packing
        assert np.all(np.diff(t0s) == 1), "non-contiguous valid tokens"
        base = int(t0s[0])
        counts = np.bincount(w)
        assert counts.max() <= TC, "word with more tokens than a chunk"
        nwords = len(counts)
        chunks = []
        i = 0
        toff = 0
        while i < nwords:
            ntok = 0
            nw = 0
            while (
                i + nw < nwords and nw < WC and ntok + counts[i + nw] <= TC
            ):
                ntok += int(counts[i + nw])
                nw += 1
            chunks.append(
                {"sample": b, "t0": base + toff, "ntok": ntok, "w0": i, "nw": nw}
            )
            toff += ntok
            i += nw
        chunks_by_sample.append(chunks)
    order = sorted(range(B), key=lambda b: -len(chunks_by_sample[b]))
    percore = []
    for c in range(N_CORES):
        bs = [order[c], order[2 * N_CORES - 1 - c]]
        ch = []
        for b in bs:
            ch.extend(chunks_by_sample[b])
        percore.append(ch)
    nct_raw = max(len(ch) for ch in percore)
    nct = -(-nct_raw // pad_mult) * pad_mult
    pad = {"sample": -1, "t0": 0, "ntok": 0, "w0": 0, "nw": 0}
    for ch in percore:
        ch.extend([pad] * (nct - len(ch)))
    return percore, nct


OUT_F8_SCALE = 2.0  # output pre-scale when writing f8e3 (undone on host);
# e3m4 max normal is 15.5 and max |word mean| here is ~5.7, so 2.0 keeps
# headroom while pushing small values above the subnormal floor


def _prep_inputs2(hidden_states, word_ids, percore, nct, in_dtype="f8e3",
                  in_group=8, out_dtype="f16"):
    import ml_dtypes

    hs = np.asarray(hidden_states, dtype=np.float32)
    wid = np.minimum(np.asarray(word_ids, dtype=np.int32), NUM_WORDS - 1)
    if in_dtype == "f8e3":
        hs8 = (hs * FP8_SCALE).astype(ml_dtypes.float8_e3m4)
        descale = 1.0 / FP8_SCALE
    else:
        hs8 = hs.astype(np.float16 if in_dtype == "f16" else np.float32)
        descale = 1.0
    oscale = OUT_F8_SCALE if out_dtype == "f8e3" else 1.0
    ndual = nct // 2
    in_maps = []
    for c in range(N_CORES):
        ch = percore[c]
        x = np.zeros((nct, P, H), hs8.dtype)
        widrel = np.full((P, nct), -1.0, np.float32)
        rcp = np.zeros((P, ndual), np.float32)
        for ci, k in enumerate(ch):
            n = k["ntok"]
            if n == 0:
                continue
            b, t0 = k["sample"], k["t0"]
            x[ci, :n] = hs8[b, t0 : t0 + n]
            wr = wid[b, t0 : t0 + n] - k["w0"]
            widrel[:n, ci] = wr.astype(np.float32)
            cnt = np.bincount(wr, minlength=k["nw"]).astype(np.float32)
            half = ci % 2
            rcp[half * WC : half * WC + k["nw"], ci // 2] = (
                oscale * descale / cnt[: k["nw"]]
            )
        xg = np.ascontiguousarray(
            x.reshape(nct // in_group, in_group, P, H)
            .transpose(0, 2, 1, 3)
            .reshape(nct // in_group, P, in_group * H)
        )
        meta = np.ascontiguousarray(np.concatenate([widrel, rcp], axis=1))
        in_maps.append({"x": xg, "meta": meta})
    return in_maps


def _unshard2(res_y, percore, nct, out_dtype="f16"):
    """Device outputs [ndual*? ...] per core -> full [B, NUM_WORDS, H]."""
    out = np.zeros((B, NUM_WORDS, H), np.float32)
    oscale = OUT_F8_SCALE if out_dtype == "f8e3" else 1.0
    for c in range(N_CORES):
        yc = np.asarray(res_y[c], dtype=np.float32) / oscale
        # yc shape [ndual//OG, P, OG*H] -> [ndual, P, H] word-chunk-major
        ndual = nct // 2
        og = yc.shape[2] // H
        yd = yc.reshape(ndual // og, P, og, H).transpose(0, 2, 1, 3).reshape(
            ndual, P, H
        )
        for ci, k in enumerate(percore[c]):
            if k["ntok"] == 0:
                continue
            half = ci % 2
            rows = yd[ci // 2, half * WC : half * WC + k["nw"]]
            out[k["sample"], k["w0"] : k["w0"] + k["nw"]] = rows
    return out


def _build2(
    nct,
    reps=1,
    dyn_reps=1,
    in_group=8,
    out_group=4,
    in_dtype="f8e3",
    out_dtype="f16",
    in_alt=False,
    out_engine="scalar",
    oh_engine="vector",
    ev_engine="alt",
    x_bufs=5,
    oh_bufs=6,
    ev_bufs=4,
    ps_bufs=4,
    mm_order="interleave",
    oh_mode="per_chunk",
    ps_pair=False,
    hints=False,
    do_in=True,
    do_mm=True,
    do_ev=True,
    do_out=True,
):
    from contextlib import nullcontext
    import concourse.bacc as bacc
    import concourse.tile as tile
    from concourse import mybir

    nc = bacc.Bacc(
        "TRN2",
        target_bir_lowering=False,
        debug=False,
        enable_asserts=False,
        num_devices=N_CORES,
    )
    f32 = mybir.dt.float32
    fin = {"f16": mybir.dt.float16, "f8e3": mybir.dt.float8e3, "f32": f32}[in_dtype]
    fout = {"f16": mybir.dt.float16, "f8e3": mybir.dt.float8e3, "f32": f32}[
        out_dtype
    ]
    IG, OG = in_group, out_group
    assert nct % (2 * OG) == 0 and nct % IG == 0 and IG % 2 == 0
    if ps_pair:
        assert OG == 2, "ps_pair requires out_group=2 (one ev tile per pair)"
        ps_bufs = min(ps_bufs, 2)  # [P, 2*H] f32 = 3 banks; 2 bufs = 6 of 8
    NDUAL = nct // 2
    NG = nct // IG
    x = nc.dram_tensor("x", [NG, P, IG * H], fin, kind="ExternalInput").ap()
    meta = nc.dram_tensor("meta", [P, nct + NDUAL], f32, kind="ExternalInput").ap()
    y = nc.dram_tensor(
        "y", [NDUAL // OG, P, OG * H], fout, kind="ExternalOutput"
    ).ap()

    with tile.TileContext(nc) as tc:
        with (
            tc.tile_pool(name="const", bufs=1) as const_pool,
            tc.tile_pool(name="xin", bufs=x_bufs) as x_pool,
            tc.tile_pool(name="oh", bufs=oh_bufs) as oh_pool,
            tc.tile_pool(name="ev", bufs=ev_bufs) as ev_pool,
            tc.tile_pool(name="psum", bufs=ps_bufs, space="PSUM") as psum_pool,
        ):
            iota_f = const_pool.tile([P, WC], f32)
            nc.gpsimd.iota(
                iota_f[:], pattern=[[1, WC]], base=0, channel_multiplier=0,
                allow_small_or_imprecise_dtypes=True,
            )
            meta_t = const_pool.tile([P, nct + NDUAL], f32, name="meta")
            nc.scalar.dma_start(out=meta_t[:], in_=meta[:, :])
            wid_t = meta_t[:, :nct]
            rcp_t = meta_t[:, nct:]

            oh_eng = nc.gpsimd if oh_engine == "gpsimd" else nc.vector

            def dma_eng(g):
                if in_alt == "gpsimd":
                    return nc.gpsimd if g % 2 == 1 else nc.sync
                if in_alt == "3way":
                    return (nc.sync, nc.scalar, nc.gpsimd)[g % 3]
                if in_alt:
                    return nc.scalar if g % 2 == 1 else nc.sync
                return nc.sync

            def emit(rep):
                xg_tiles = {}
                ohg_tiles = {}
                og_tile = [None]
                pair_ps = [None]
                for dd in range(NDUAL):
                    c0, c1 = 2 * dd, 2 * dd + 1
                    g = c0 // IG
                    if g not in xg_tiles:
                        xt = x_pool.tile(
                            [P, IG, H], fin, name=f"xt_{rep}_{g}", tag="xt"
                        )
                        if do_in:
                            src = x[g, :, :].rearrange("p (a h) -> p a h", a=IG)
                            dma_eng(g).dma_start(out=xt[:], in_=src)
                        xg_tiles[g] = xt
                        xg_tiles.pop(g - x_bufs + 1, None)
                    xt0 = xg_tiles[g][:, c0 % IG, :]
                    xt1 = xg_tiles[g][:, c1 % IG, :]
                    if oh_mode == "mega":
                        # one is_equal builds one-hots for all IG chunks of
                        # the group: iota broadcast against per-chunk wid
                        if g not in ohg_tiles:
                            ohg = oh_pool.tile(
                                [P, IG, WC], fin, name=f"oh_{rep}_{g}", tag="oh"
                            )
                            in0 = (
                                wid_t[:, g * IG : (g + 1) * IG]
                                .unsqueeze(2)
                                .broadcast_to([P, IG, WC])
                            )
                            in1 = iota_f[:].unsqueeze(1).broadcast_to(
                                [P, IG, WC]
                            )
                            oh_eng.tensor_tensor(
                                out=ohg[:], in0=in0, in1=in1,
                                op=mybir.AluOpType.is_equal,
                            )
                            ohg_tiles[g] = ohg
                            ohg_tiles.pop(g - oh_bufs + 1, None)
                        ohg = ohg_tiles[g]
                        oh0 = ohg[:, c0 % IG, :]
                        oh1 = ohg[:, c1 % IG, :]
                    else:
                        oh = oh_pool.tile(
                            [P, 2 * WC], fin, name=f"oh_{rep}_{dd}", tag="oh"
                        )
                        build = oh_eng.tensor_scalar
                        build(
                            out=oh[:, :WC], in0=iota_f[:],
                            scalar1=wid_t[:, c0 : c0 + 1],
                            scalar2=None, op0=mybir.AluOpType.is_equal,
                        )
                        build(
                            out=oh[:, WC:], in0=iota_f[:],
                            scalar1=wid_t[:, c1 : c1 + 1],
                            scalar2=None, op0=mybir.AluOpType.is_equal,
                        )
                        oh0 = oh[:, :WC]
                        oh1 = oh[:, WC:]
                    if do_mm:
                        if ps_pair:
                            # one [P, 2*H] psum tile (3 banks) per 2 duals;
                            # split ranges keep every matmul within a bank
                            j = dd % 2
                            if j == 0:
                                ps_t = psum_pool.tile(
                                    [P, 2 * H], f32,
                                    name=f"ps_{rep}_{dd // 2}", tag="ps",
                                )
                                pair_ps[0] = ps_t
                            ps_t = pair_ps[0]
                            splits = (
                                ((0, 512), (512, H)) if j == 0
                                else ((0, 256), (256, H))
                            )
                            base = j * H
                            mms = []
                            for lo, hi in splits:
                                mms.append((
                                    ps_t[0:WC, base + lo : base + hi], oh0,
                                    xt0[:, lo:hi], (0, 0)))
                                mms.append((
                                    ps_t[WC:P, base + lo : base + hi], oh1,
                                    xt1[:, lo:hi], (0, 64)))
                        else:
                            ps = psum_pool.tile(
                                [P, H], f32, name=f"ps_{rep}_{dd}", tag="ps"
                            )
                            mms = []
                            for lo, hi in NSPLITS:
                                mms.append((ps[0:WC, lo:hi], oh0,
                                            xt0[:, lo:hi], (0, 0)))
                                mms.append((ps[WC:P, lo:hi], oh1,
                                            xt1[:, lo:hi], (0, 64)))
                        if mm_order == "chunk":
                            mms = [mms[0], mms[2], mms[1], mms[3]]
                        for out_ap, lhsT, rhs, tp in mms:
                            nc.tensor.matmul(
                                out=out_ap, lhsT=lhsT, rhs=rhs,
                                start=True, stop=True, tile_position=tp,
                            )
                    if dd % OG == 0:
                        og_tile[0] = ev_pool.tile(
                            [P, OG, H], fout, name=f"out_{rep}_{dd}", tag="out"
                        )
                    out_sb = og_tile[0][:, dd % OG, :]
                    if do_mm and do_ev and ps_pair:
                        # evict once per pair tile: alternate one wide
                        # vector tensor_tensor (rcp broadcast over H) with
                        # two scalar muls
                        if dd % 2 == 1:
                            ps_t = pair_ps[0]
                            k = dd // 2
                            if k % 2 == 0:
                                src = ps_t[:].rearrange(
                                    "p (a h) -> p a h", a=2
                                )
                                rcpb = (
                                    rcp_t[:, dd - 1 : dd + 1]
                                    .unsqueeze(2)
                                    .broadcast_to([P, 2, H])
                                )
                                nc.vector.tensor_tensor(
                                    out=og_tile[0][:], in0=src, in1=rcpb,
                                    op=mybir.AluOpType.mult,
                                )
                            else:
                                for j in (0, 1):
                                    nc.scalar.mul(
                                        out=og_tile[0][:, j, :],
                                        in_=ps_t[:, j * H : (j + 1) * H],
                                        mul=rcp_t[:, dd - 1 + j : dd + j],
                                    )
                    elif do_mm and do_ev:
                        if ev_engine == "alt":
                            eng = "scalar" if dd % 2 == 0 else "vector"
                        elif ev_engine == "alt2":
                            eng = "scalar" if dd % 3 == 0 else "vector"
                        else:
                            eng = ev_engine
                        if eng == "vector":
                            nc.vector.tensor_scalar(
                                out=out_sb, in0=ps[:], scalar1=rcp_t[:, dd : dd + 1],
                                scalar2=None, op0=mybir.AluOpType.mult,
                            )
                        else:
                            nc.scalar.mul(
                                out=out_sb, in_=ps[:], mul=rcp_t[:, dd : dd + 1]
                            )
                    elif do_ev:
                        nc.gpsimd.memset(out_sb, 0.0)
                    if do_out and dd % OG == OG - 1:
                        dst = y[dd // OG, :, :].rearrange("p (a h) -> p a h", a=OG)
                        if out_engine == "alt":
                            oe = nc.scalar if (dd // OG) % 2 == 0 else nc.sync
                        elif out_engine == "gpsimd":
                            oe = nc.gpsimd
                        else:
                            oe = nc.scalar
                        oe.dma_start(out=dst, in_=og_tile[0][:])

            fkw = {}
            if hints and dyn_reps > 1:
                fkw = dict(
                    back_edge_label="bh", hint_engines=list(mybir.ALL_ENGINES)
                )
            loop_cm = (
                tc.For_i(0, dyn_reps, 1, **fkw) if dyn_reps > 1 else nullcontext()
            )
            with loop_cm:
                if hints and dyn_reps > 1:
                    tc.mark_branch_hint_location(
                        "bh", engines=list(mybir.ALL_ENGINES)
                    )
                for rep in range(reps):
                    emit(rep)

    nc.compile()
    return nc


BEST2 = {
    "in_group": 4,
    "out_group": 2,
    "in_dtype": "f8e3",
    "out_dtype": "f8e3",
    "in_alt": False,
    "oh_engine": "vector",
    "ev_engine": "alt",
    "x_bufs": 8,
    "oh_bufs": 10,
    "ev_bufs": 6,
}
R_UNROLL = 32  # bodies per hardware-loop iteration for steady-state timing


def _prep_kwargs2(bkw):
    return {
        "in_dtype": bkw.get("in_dtype", "f8e3"),
        "in_group": bkw.get("in_group", 8),
        "out_dtype": bkw.get("out_dtype", "f16"),
    }


def kernel_v2(hidden_states, word_ids):
    import concourse.bass_utils as bass_utils

    wid = np.asarray(word_ids, dtype=np.int32)
    percore, nct = _plan2(wid, pad_mult=_pad_mult2(BEST2))
    nc = _build2(nct, **BEST2)
    in_maps = _prep_inputs2(hidden_states, word_ids, percore, nct,
                            **_prep_kwargs2(BEST2))
    res = bass_utils.run_bass_kernel_spmd(nc, in_maps, core_ids=list(range(N_CORES)))
    ys = [res.results[c]["y"] for c in range(N_CORES)]
    return _unshard2(ys, percore, nct, out_dtype=BEST2.get("out_dtype", "f16"))


def _pad_mult2(bkw):
    ig = bkw.get("in_group", 8)
    og = bkw.get("out_group", 4)
    import math

    return math.lcm(ig, 2 * og)


def _prep_inputs(hidden_states, word_ids, perm=None, in_dtype="f16",
                 in_layout="rowmajor", in_group=4, rcp_mode="onehot"):
    hs = np.asarray(hidden_states, dtype=np.float32)
    if in_dtype == "f8e3":
        import ml_dtypes

        hs = np.ascontiguousarray((hs * FP8_SCALE).astype(ml_dtypes.float8_e3m4))
        descale = 1.0 / FP8_SCALE
    else:
        np_in = np.float16 if in_dtype == "f16" else np.float32
        hs = np.ascontiguousarray(hs.astype(np_in))
        descale = 1.0
    wid = np.minimum(np.asarray(word_ids, dtype=np.int32), NUM_WORDS - 1)
    assert hs.shape == (B, S, H) and wid.shape == (B, S)
    if perm is None:
        perm = np.arange(B, dtype=np.int64).reshape(N_CORES, SPC)
    # [B, S] -> [B, P, KT]: element (p, t) = token t*P + p
    widf = np.ascontiguousarray(
        wid.astype(np.float32).reshape(B, KT, P).transpose(0, 2, 1)
    )
    if rcp_mode == "evict":
        # per-word 1/count in [B, P, NW]: element (p, j) = word j*P + p
        rt = np.zeros((B, NUM_WORDS), np.float32)
        for b in range(B):
            w = wid[b]
            counts = np.bincount(w[w >= 0], minlength=NUM_WORDS)
            nz = counts > 0
            rt[b, nz] = descale / counts[nz]
        rt = np.ascontiguousarray(rt.reshape(B, NW, P).transpose(0, 2, 1))
    else:
        r = _recip_counts(wid)
        rt = np.ascontiguousarray(r.reshape(B, KT, P).transpose(0, 2, 1))
    in_maps = []
    for c in range(N_CORES):
        sl = list(perm[c])
        if in_layout == "pmajor":
            IG = in_group
            xc = np.ascontiguousarray(
                hs[sl]
                .reshape(SPC, KT // IG, IG, P, H)
                .transpose(0, 1, 3, 2, 4)
                .reshape(SPC * KT // IG, P, IG * H)
            )
        elif in_layout == "pmajor_fused":
            IG = in_group
            xc = np.ascontiguousarray(
                hs[sl]
                .reshape(SPC, KT // IG, IG, P, H)
                .transpose(1, 3, 0, 2, 4)  # [KT//IG, P, SPC, IG, H]
                .reshape(KT // IG, P, SPC * IG * H)
            )
        else:
            xc = np.ascontiguousarray(hs[sl].reshape(SPC * S, H))
        mc = np.ascontiguousarray(
            np.concatenate(
                [np.concatenate([widf[s], rt[s]], axis=1) for s in sl], axis=1
            )
        )
        in_maps.append({"x": xc, "meta": mc})
    return in_maps


# Best-known configuration (applied by kernel(); bench.py overrides).
# fp8-e3m4 input (host pre-scaled by 2, descaled via the per-word 1/count
# applied at PSUM eviction, which keeps the one-hot exactly representable):
# end-to-end rel err 1.33e-2 vs the 2e-2 gate; halves input HBM traffic.
BEST = {
    "in_layout": "pmajor_fused",
    "in_group": 4,
    "out_layout": "pmajor",
    "out_group": 4,
    "rcp_mode": "evict",
    "in_dtype": "f8e3",
}


def _prep_kwargs(bkw):
    return {
        "in_dtype": bkw.get("in_dtype", "f16"),
        "in_layout": bkw.get("in_layout", "rowmajor"),
        "in_group": bkw.get("in_group", 4),
        "rcp_mode": bkw.get("rcp_mode", "onehot"),
    }


def _unshard(yc, bkw):
    """Per-core device output -> [SPC, NUM_WORDS, H]."""
    if bkw.get("out_layout", "rowmajor") == "pmajor":
        og = bkw.get("out_group", 2)
        return (
            yc.reshape(SPC, NW // og, P, og, H)
            .transpose(0, 1, 3, 2, 4)
            .reshape(SPC, NUM_WORDS, H)
        )
    return yc.reshape(SPC, NUM_WORDS, H)


def kernel_v1(hidden_states, word_ids):
    import concourse.bass_utils as bass_utils

    wid = np.asarray(word_ids, dtype=np.int32)
    plans, perm = _plan(wid)
    nc = _build(plans, **BEST)
    in_maps = _prep_inputs(hidden_states, word_ids, perm, **_prep_kwargs(BEST))
    res = bass_utils.run_bass_kernel_spmd(nc, in_maps, core_ids=list(range(N_CORES)))
    out = np.empty((B, NUM_WORDS, H), np.float32)
    for c in range(N_CORES):
        yc = np.asarray(res.results[c]["y"], dtype=np.float32)
        yb = _unshard(yc, BEST)
        for slot in range(SPC):
            out[perm[c][slot]] = yb[slot]
    return out


def kernel(hidden_states, word_ids):
    return kernel_v2(hidden_states, word_ids)



# revision 23
# speedup vs baseline: 1.0665x; 1.0665x over previous
"""Segment-mean (word-pooling) kernel for Trainium2, 8 NeuronCores.

Problem: hidden_states [16, 4096, 768] f32, word_ids [16, 4096] i32
(non-decreasing per row, -1 = special token). Output [16, 2048, 768] f32:
mean of each word's subword embeddings; words with no tokens -> 0.

Active design (V2, word-aligned chunking; the older V1 banded-window
formulation is kept below for reference):

Pure data parallelism, 2 samples per core. The valid-token stream of
each sample (positions 1..4094 here; -1 only marks CLS/SEP) is chunked
host-side into pieces of <=128 tokens covering <=64 WHOLE words — no
word ever straddles a chunk — so each chunk needs exactly ONE one-hot
matmul (K=128 tokens, M=64 words, N=768) into its own psum half:
  ps[w, h] = sum_tok onehot[tok, w] * x[tok, h]
Two chunks pair into one [128, 768] psum tile via PE column tiling
(tile_position (0,0)/(0,64)); the per-word 1/count (host-computed,
carries the fp8 descale and f8 output scale) is applied during the
PSUM->SBUF eviction, alternated between the scalar and vector engines.
All data-dependence (chunk boundaries) lives in the host-packed x/meta
tensors; the device program is completely data-independent, so the SPMD
program is trivially identical across cores (samples are paired to
cores largest+smallest to balance chunk counts).

Memory-path choices, all validated on HW:
  - x is fp8-e3m4 (host pre-scaled by 2, descale folded into 1/count);
    one-hots stay exact in fp8 (pure 0/1). Output is ALSO fp8-e3m4
    (host pre-scaled by 2): end-to-end rel err 1.525e-2 vs the 2e-2
    gate. Per-core HBM traffic: 6.68 MB in + 3.34 MB out ~= the 28 us
    HBM-per-core roofline (358 GB/s), which measurement shows is the
    binding resource - the body runs within ~4 us of it.
  - input ships chunk-major [nchunks, 128, 768] so every DMA line is
    one contiguous multi-KB chunk per partition; input DMAs ride the
    sync HWDGE ring, output the scalar ring. Splitting input DMAs
    across both rings measured WORSE (the scalar engine's strict-FIFO
    queue delays DMA issue behind evictions).
  - measured on this stack: a hardware For_i iteration carries a fixed
    ~46 us reset/barrier cost even with an EMPTY body, so steady-state
    timing (test.py) unrolls R_UNROLL kernel bodies per loop iteration
    and reports the marginal per-body slope.
"""

import numpy as np

B, S, H = 16, 4096, 768
NUM_WORDS = S // 2  # 2048
N_CORES = 8
SPC = B // N_CORES  # samples per core = 2
P = 128
KT = S // P  # 32 k-tiles per sample
NW = NUM_WORDS // P  # 16 output windows per sample
NSPLITS = ((0, 512), (512, 768))  # matmul free-dim splits of H


def _plan(word_ids: np.ndarray):
    """Per-slot union plan. Samples are assigned to slots to minimize the
    total union window-span (= matmul pairs): for each slot and k-tile t
    the span [minwin, maxwin] over that slot's 8 samples; per window j
    the sorted member k-tiles. Returns (plans, perm) where plans is
    (spans, members) per slot and perm[c][s] = sample of core c, slot s."""
    word_ids = np.minimum(word_ids, NUM_WORDS - 1)
    minw = np.full((B, KT), NW, np.int64)
    maxw = np.full((B, KT), -1, np.int64)
    for b in range(B):
        row = word_ids[b]
        for t in range(KT):
            w = row[t * P : (t + 1) * P]
            w = w[w >= 0]
            if w.size:
                minw[b, t] = w.min() // P
                maxw[b, t] = w.max() // P
    if SPC == 2:
        from itertools import combinations

        allb = set(range(B))
        best = None
        for g0r in combinations(range(1, B), N_CORES - 1):
            g0 = [0, *g0r]
            g1 = sorted(allb - set(g0))
            cost = 0
            for g in (g0, g1):
                cost += int(
                    np.sum(maxw[g].max(0) - np.minimum(minw[g].min(0), maxw[g].max(0)) + 1)
                )
            if best is None or cost < best[0]:
                best = (cost, g0, g1)
        groups = [best[1], best[2]]
    else:
        groups = [list(range(s, B, SPC)) for s in range(SPC)]
    perm = np.array(groups, dtype=np.int64).T  # [N_CORES, SPC]
    plans = []
    for g in groups:
        mn = np.minimum(minw[g].min(0), maxw[g].max(0))
        mx = maxw[g].max(0)
        members = {j: [] for j in range(NW)}
        spans = []
        for t in range(KT):
            spans.append((int(mn[t]), int(mx[t])))
            for j in range(int(mn[t]), int(mx[t]) + 1):
                members[j].append(t)
        plans.append((spans, members))
    return plans, perm


def _liveness(plans, in_group):
    """Max number of simultaneously-live x DMA groups / onehot tiles over
    the per-window emission order, across slots. A pool needs at least
    this many bufs or slot reuse can deadlock the DMA ring."""
    max_live_g, max_live_oh = 0, 0
    for spans, members in plans:
        first_g, last_g, first_oh, last_oh = {}, {}, {}, {}
        for j in range(NW):
            for t in members[j]:
                g = t // in_group
                first_g.setdefault(g, j)
                last_g[g] = j
                first_oh.setdefault(t, j)
                last_oh[t] = j
        for j in range(NW):
            live_g = sum(1 for g in first_g if first_g[g] <= j <= last_g[g])
            live_oh = sum(1 for t in first_oh if first_oh[t] <= j <= last_oh[t])
            max_live_g = max(max_live_g, live_g)
            max_live_oh = max(max_live_oh, live_oh)
    return max_live_g, max_live_oh


def _recip_counts(word_ids: np.ndarray) -> np.ndarray:
    """Per-token 1/count(word) as f32; 0 for special (-1) tokens."""
    r = np.zeros((B, S), np.float32)
    for b in range(B):
        wid = word_ids[b]
        valid = wid >= 0
        counts = np.bincount(wid[valid], minlength=NUM_WORDS)
        r[b, valid] = (1.0 / counts[wid[valid]]).astype(np.float32)
    return r


def _build(
    plans,
    reps=1,
    dyn_reps=1,
    do_mm=True,
    do_out=True,
    do_in=True,
    x_bufs=8,
    oh_bufs=8,
    ev_bufs=4,
    ps_bufs=3,
    in_group=4,
    out_group=2,
    in_alt=False,
    out_engine="scalar",
    in_dtype="f16",
    out_dtype="f16",
    n_splits=None,
    ev_engine="scalar",
    in_layout="rowmajor",
    out_layout="rowmajor",
    rcp_mode="onehot",
    split_psum=False,
    do_ev=True,
    oh_const=False,
    interleave=False,
):
    """Build + compile the SPMD Bass program. reps>1 unrolls the whole
    body; dyn_reps>1 wraps it in a hardware For loop — both only used
    for amortized wall-clock timing. do_* flags ablate kernel stages
    for benchmarking (outputs are wrong when any is False)."""
    from contextlib import nullcontext
    import concourse.bacc as bacc
    import concourse.tile as tile
    from concourse import mybir

    nc = bacc.Bacc(
        "TRN2",
        target_bir_lowering=False,
        debug=False,
        enable_asserts=False,
        num_devices=N_CORES,
    )
    f32 = mybir.dt.float32
    fin = {"f16": mybir.dt.float16, "f8e3": mybir.dt.float8e3, "f32": f32}[in_dtype]
    fout = mybir.dt.float16 if out_dtype == "f16" else f32
    assert rcp_mode in ("onehot", "evict")
    if in_dtype == "f8e3":
        assert rcp_mode == "evict", "fp8 one-hot cannot carry 1/count exactly"
    fused = in_layout == "pmajor_fused"
    if in_layout == "pmajor":
        x = nc.dram_tensor(
            "x", [SPC * KT // in_group, P, in_group * H], fin, kind="ExternalInput"
        ).ap()
    elif fused:
        # both slots' k-tile group g in one DMA: halves dma_start count
        x = nc.dram_tensor(
            "x", [KT // in_group, P, SPC * in_group * H], fin, kind="ExternalInput"
        ).ap()
    else:
        x = nc.dram_tensor("x", [SPC * S, H], fin, kind="ExternalInput").ap()
    # onehot mode: per-token 1/count [.., KT]; evict mode: per-word [.., NW];
    # word ids and 1/count ship fused in one tensor -> one prologue DMA
    rcp_w = KT if rcp_mode == "onehot" else NW
    meta = nc.dram_tensor(
        "meta", [P, SPC * (KT + rcp_w)], f32, kind="ExternalInput"
    ).ap()
    if out_layout == "pmajor":
        y = nc.dram_tensor(
            "y", [SPC * (NW // out_group), P, out_group * H], fout,
            kind="ExternalOutput",
        ).ap()
    else:
        y = nc.dram_tensor(
            "y", [SPC * NUM_WORDS, H], fout, kind="ExternalOutput"
        ).ap()

    IOTA_W = NUM_WORDS + 2 * P  # ramp long enough for any window pair
    max_span = max(
        (jhi - jlo + 1) for spans, _ in plans for (jlo, jhi) in spans
    )

    # Size pools from plan liveness; degenerate plans (heavily overlapping
    # window k-ranges) fall back to a bounded reload mode, otherwise pool
    # slot reuse can deadlock the DMA ring.
    in_b = {"f16": 2, "f8e3": 1, "f32": 4}[in_dtype]
    live_g, live_oh = _liveness(plans, in_group)
    if fused:
        # every fused group tile stays live from slot 0 use to slot 1 use
        need_x, need_oh = KT // in_group + 2, live_oh + 3
        x_bytes = need_x * SPC * in_group * H * in_b
    else:
        need_x, need_oh = live_g + 3, live_oh + 3
        x_bytes = need_x * in_group * H * in_b
    oh_bytes = need_oh * max_span * P * in_b
    safe = x_bytes + oh_bytes > 150 * 1024
    if not safe:
        x_bufs = max(x_bufs, need_x)
        oh_bufs = max(oh_bufs, need_oh)
    assert not (fused and safe), "fused input layout incompatible with safe mode"

    with tile.TileContext(nc) as tc:
        with (
            tc.tile_pool(name="const", bufs=1) as const_pool,
            tc.tile_pool(name="xin", bufs=x_bufs) as x_pool,
            tc.tile_pool(name="oh", bufs=oh_bufs) as oh_pool,
            tc.tile_pool(name="ev", bufs=ev_bufs) as ev_pool,
            tc.tile_pool(
                name="psum", bufs=(4 if split_psum else ps_bufs), space="PSUM"
            ) as psum_pool,
            tc.tile_pool(name="psumb", bufs=4, space="PSUM") as psumb_pool,
        ):
            # f32 iota is exact for our word-index range (< 2^24): build the
            # ramp directly, no int staging tile + DVE copy in the prologue
            iota_f = const_pool.tile([P, IOTA_W], f32)
            nc.gpsimd.iota(
                iota_f[:], pattern=[[1, IOTA_W]], base=0, channel_multiplier=0,
                allow_small_or_imprecise_dtypes=True,
            )
            oh_c = None
            if oh_const:  # ablation: one fixed oh tile, no per-tile DVE builds
                oh_c = const_pool.tile([P, P], fin, name="oh_const")
                nc.vector.memset(oh_c[:], 0.0)

            IG, OG = in_group, out_group
            out_eng = nc.sync if out_engine == "sync" else nc.scalar
            ev_eng = nc.vector if ev_engine == "vector" else nc.scalar
            splits = NSPLITS if n_splits is None else n_splits

            # wid/rcp are loop constants: ONE fused DMA, outside any reps loop
            SW = KT + rcp_w
            meta_t = const_pool.tile([P, SPC * SW], f32, name="meta")
            nc.scalar.dma_start(out=meta_t[:], in_=meta[:, :])
            wid_ts = [meta_t[:, s * SW : s * SW + KT] for s in range(SPC)]
            rcp_ts = [meta_t[:, s * SW + KT : (s + 1) * SW] for s in range(SPC)]

            def emit(rep):
                fused_tiles = {}

                def slot_body(slot):
                    spans, members = plans[slot]
                    wid_t = wid_ts[slot]
                    rcp_t = rcp_ts[slot]

                    def build_oh(out_ap, iota_ap, t):
                        if rcp_mode == "onehot":
                            nc.vector.tensor_scalar(
                                out=out_ap,
                                in0=iota_ap,
                                scalar1=wid_t[:, t : t + 1],
                                scalar2=rcp_t[:, t : t + 1],
                                op0=mybir.AluOpType.is_equal,
                                op1=mybir.AluOpType.mult,
                            )
                        else:
                            nc.vector.tensor_scalar(
                                out=out_ap,
                                in0=iota_ap,
                                scalar1=wid_t[:, t : t + 1],
                                scalar2=None,
                                op0=mybir.AluOpType.is_equal,
                            )

                    def evict(out_ap, ps_ap, j, eng):
                        if rcp_mode == "onehot":
                            if eng == "vector":
                                nc.vector.tensor_copy(out=out_ap, in_=ps_ap)
                            else:
                                nc.scalar.copy(out=out_ap, in_=ps_ap)
                        else:
                            if eng == "vector":
                                nc.vector.tensor_scalar(
                                    out=out_ap,
                                    in0=ps_ap,
                                    scalar1=rcp_t[:, j : j + 1],
                                    scalar2=None,
                                    op0=mybir.AluOpType.mult,
                                )
                            else:
                                nc.scalar.mul(
                                    out=out_ap, in_=ps_ap, mul=rcp_t[:, j : j + 1]
                                )

                    xg_tiles = {}
                    oh_tiles = {}

                    def dma_eng(g):
                        if in_alt == "gpsimd":
                            return nc.gpsimd if g % 2 == 1 else nc.sync
                        if in_alt == "3way":
                            return (nc.sync, nc.scalar, nc.gpsimd)[g % 3]
                        if in_alt:
                            return nc.scalar if g % 2 == 1 else nc.sync
                        return nc.sync

                    def get_x(t):
                        g, a = divmod(t, IG)
                        if fused:
                            if g not in fused_tiles:
                                xt = x_pool.tile(
                                    [P, SPC, IG, H], fin,
                                    name=f"xt_{rep}_{g}", tag="xt",
                                )
                                if do_in:
                                    src = x[g, :, :].rearrange(
                                        "p (s a h) -> p s a h", s=SPC, a=IG
                                    )
                                    dma_eng(g).dma_start(out=xt[:], in_=src)
                                fused_tiles[g] = xt
                            return fused_tiles[g][:, slot, a, :]
                        if g not in xg_tiles:
                            xt = x_pool.tile(
                                [P, IG, H], fin, name=f"xt_{rep}_{slot}_{g}", tag="xt"
                            )
                            if do_in:
                                if in_layout == "pmajor":
                                    src = x[slot * (KT // IG) + g, :, :].rearrange(
                                        "p (a h) -> p a h", a=IG
                                    )
                                else:
                                    r0 = slot * S + g * IG * P
                                    src = x[r0 : r0 + IG * P, :].rearrange(
                                        "(a p) h -> p a h", p=P
                                    )
                                dma_eng(g).dma_start(out=xt[:], in_=src)
                            xg_tiles[g] = xt
                        return xg_tiles[g][:, t % IG, :]

                    def get_oh(t):
                        if oh_const:
                            return oh_c
                        if t not in oh_tiles:
                            jlo, jhi = spans[t]
                            wspan = (jhi - jlo + 1) * P
                            oh = oh_pool.tile(
                                [P, max_span * P],
                                fin,
                                name=f"oh_{rep}_{slot}_{t}",
                                tag="oh",
                            )
                            build_oh(
                                oh[:, :wspan],
                                iota_f[:, jlo * P : jlo * P + wspan],
                                t,
                            )
                            oh_tiles[t] = oh
                        return oh_tiles[t]

                    og_tile = [None]

                    for j in range(NW):
                        if j % OG == 0:
                            og_tile[0] = ev_pool.tile(
                                [P, OG, H], fout, name=f"out_{rep}_{slot}_{j}", tag="out"
                            )
                        out_sb = og_tile[0][:, j % OG, :]
                        ks = members[j]
                        if not do_mm:
                            for t in ks:
                                get_x(t)
                                get_oh(t)
                        if not ks:
                            nc.vector.memset(out_sb, 0.0)
                        elif not do_mm:
                            if do_out:
                                nc.gpsimd.memset(out_sb, 0.0)
                        else:
                            if split_psum:
                                pss = [
                                    (psum_pool if si == 0 else psumb_pool).tile(
                                        [P, hi - lo], f32,
                                        name=f"ps{si}_{rep}_{slot}_{j}",
                                        tag=f"ps{si}",
                                    )
                                    for si, (lo, hi) in enumerate(splits)
                                ]
                            else:
                                ps = psum_pool.tile(
                                    [P, H], f32, name=f"ps_{rep}_{slot}_{j}", tag="ps"
                                )
                            for ki, t in enumerate(ks):
                                if safe:
                                    xs = x_pool.tile(
                                        [P, 1, H], fin,
                                        name=f"xs_{rep}_{slot}_{j}_{t}", tag="xt",
                                    )
                                    r0 = slot * S + t * P
                                    nc.sync.dma_start(
                                        out=xs[:], in_=x[r0 : r0 + P, :].rearrange(
                                            "(a p) h -> p a h", p=P
                                        )
                                    )
                                    xt = xs[:, 0, :]
                                    oh = oh_pool.tile(
                                        [P, P], fin,
                                        name=f"ohs_{rep}_{slot}_{j}_{t}", tag="oh",
                                    )
                                    build_oh(
                                        oh[:, :],
                                        iota_f[:, j * P : (j + 1) * P],
                                        t,
                                    )
                                    off = 0
                                else:
                                    xt = get_x(t)
                                    oh = get_oh(t)
                                    off = 0 if oh_const else (j - spans[t][0]) * P
                                for si, (lo, hi) in enumerate(splits):
                                    nc.tensor.matmul(
                                        out=(
                                            pss[si][:, :]
                                            if split_psum
                                            else ps[:, lo:hi]
                                        ),
                                        lhsT=oh[:, off : off + P],
                                        rhs=xt[:, lo:hi],
                                        start=(ki == 0),
                                        stop=(ki == len(ks) - 1),
                                    )
                            if not do_ev:
                                pass
                            elif split_psum:
                                for si, (lo, hi) in enumerate(splits):
                                    eng = (
                                        ("scalar" if si == 0 else "vector")
                                        if ev_engine == "alt"
                                        else ev_engine
                                    )
                                    evict(out_sb[:, lo:hi], pss[si][:, :], j, eng)
                            elif ev_engine == "alt":
                                evict(out_sb, ps, j, "scalar" if j % 2 == 0 else "vector")
                            else:
                                evict(out_sb, ps, j, ev_engine)
                        if do_out and j % OG == OG - 1:
                            if out_layout == "pmajor":
                                dst = y[
                                    slot * (NW // OG) + j // OG, :, :
                                ].rearrange("p (a h) -> p a h", a=OG)
                            else:
                                r0 = slot * NUM_WORDS + (j - OG + 1) * P
                                dst = y[r0 : r0 + OG * P, :].rearrange(
                                    "(a p) h -> p a h", p=P
                                )
                            if out_engine == "alt":
                                oe = nc.scalar if (j // OG) % 2 == 0 else nc.sync
                            elif out_engine == "swdge_alt":
                                oe = nc.scalar if (j // OG) % 2 == 0 else nc.gpsimd
                            else:
                                oe = out_eng
                            oe.dma_start(out=dst, in_=og_tile[0][:])
                        yield

                gens = [slot_body(s) for s in range(SPC)]
                if interleave:
                    # alternate windows across slots: two independent dep
                    # chains keep engines fed when one chain stalls
                    done = [False] * SPC
                    while not all(done):
                        for i, g in enumerate(gens):
                            if not done[i]:
                                try:
                                    next(g)
                                except StopIteration:
                                    done[i] = True
                else:
                    for g in gens:
                        for _ in g:
                            pass

            loop_cm = (
                tc.For_i(0, dyn_reps, 1) if dyn_reps > 1 else nullcontext()
            )
            with loop_cm:
                for rep in range(reps):
                    emit(rep)

    nc.compile()
    return nc


FP8_SCALE = 2.0  # host-side pre-scale for f8e3 inputs; undone via rcp


# ---------------------------------------------------------------------------
# V2: word-aligned chunked formulation.
#
# Chunk the (valid) token stream of each sample into pieces of <=TC=128
# tokens covering <=WC=64 whole words (no word straddles a chunk), so each
# chunk needs exactly ONE one-hot matmul (M=64 words, K=128 tokens, N=768)
# into its own psum half. Two chunks pair into one [128, 768] psum tile via
# PE column tiling (tile_position (0,0)/(0,64)), which the PE can run
# concurrently. Output is written chunk-major and re-assembled on host.
# All data-dependence (chunk boundaries) lives in host-packed x/meta; the
# device program is data-independent.
# ---------------------------------------------------------------------------

TC = 128  # tokens per chunk (PE contraction dim)
WC = 64  # words per chunk (half a psum tile)


def _plan2(word_ids: np.ndarray, pad_mult=8):
    """Chunk each sample; assign samples to cores pairing large with small
    chunk counts. Returns (percore, nct) where percore[c] is a list of
    chunk dicts (sample, t0, ntok, w0, nw) padded to common length nct."""
    wid = np.minimum(np.asarray(word_ids, dtype=np.int64), NUM_WORDS - 1)
    chunks_by_sample = []
    for b in range(B):
        row = wid[b]
        valid = row >= 0
        w = row[valid]
        t0s = np.nonzero(valid)[0]
        # valid tokens must be one contiguous run for slice-based packing
        assert np.all(np.diff(t0s) == 1), "non-contiguous valid tokens"
        base = int(t0s[0])
        counts = np.bincount(w)
        assert counts.max() <= TC, "word with more tokens than a chunk"
        nwords = len(counts)
        chunks = []
        i = 0
        toff = 0
        while i < nwords:
            ntok = 0
            nw = 0
            while (
                i + nw < nwords and nw < WC and ntok + counts[i + nw] <= TC
            ):
                ntok += int(counts[i + nw])
                nw += 1
            chunks.append(
                {"sample": b, "t0": base + toff, "ntok": ntok, "w0": i, "nw": nw}
            )
            toff += ntok
            i += nw
        chunks_by_sample.append(chunks)
    order = sorted(range(B), key=lambda b: -len(chunks_by_sample[b]))
    percore = []
    for c in range(N_CORES):
        bs = [order[c], order[2 * N_CORES - 1 - c]]
        ch = []
        for b in bs:
            ch.extend(chunks_by_sample[b])
        percore.append(ch)
    nct_raw = max(len(ch) for ch in percore)
    nct = -(-nct_raw // pad_mult) * pad_mult
    pad = {"sample": -1, "t0": 0, "ntok": 0, "w0": 0, "nw": 0}
    for ch in percore:
        ch.extend([pad] * (nct - len(ch)))
    return percore, nct


OUT_F8_SCALE = 2.0  # output pre-scale when writing f8e3 (undone on host);
# e3m4 max normal is 15.5 and max |word mean| here is ~5.7, so 2.0 keeps
# headroom while pushing small values above the subnormal floor


def _prep_inputs2(hidden_states, word_ids, percore, nct, in_dtype="f8e3",
                  in_group=8, out_dtype="f16"):
    import ml_dtypes

    hs = np.asarray(hidden_states, dtype=np.float32)
    wid = np.minimum(np.asarray(word_ids, dtype=np.int32), NUM_WORDS - 1)
    if in_dtype == "f8e3":
        hs8 = (hs * FP8_SCALE).astype(ml_dtypes.float8_e3m4)
        descale = 1.0 / FP8_SCALE
    else:
        hs8 = hs.astype(np.float16 if in_dtype == "f16" else np.float32)
        descale = 1.0
    oscale = OUT_F8_SCALE if out_dtype == "f8e3" else 1.0
    ndual = nct // 2
    in_maps = []
    for c in range(N_CORES):
        ch = percore[c]
        x = np.zeros((nct, P, H), hs8.dtype)
        widrel = np.full((P, nct), -1.0, np.float32)
        rcp = np.zeros((P, ndual), np.float32)
        for ci, k in enumerate(ch):
            n = k["ntok"]
            if n == 0:
                continue
            b, t0 = k["sample"], k["t0"]
            x[ci, :n] = hs8[b, t0 : t0 + n]
            wr = wid[b, t0 : t0 + n] - k["w0"]
            widrel[:n, ci] = wr.astype(np.float32)
            cnt = np.bincount(wr, minlength=k["nw"]).astype(np.float32)
            half = ci % 2
            rcp[half * WC : half * WC + k["nw"], ci // 2] = (
                oscale * descale / cnt[: k["nw"]]
            )
        xg = np.ascontiguousarray(
            x.reshape(nct // in_group, in_group, P, H)
            .transpose(0, 2, 1, 3)
            .reshape(nct // in_group, P, in_group * H)
        )
        meta = np.ascontiguousarray(np.concatenate([widrel, rcp], axis=1))
        in_maps.append({"x": xg, "meta": meta})
    return in_maps


def _unshard2(res_y, percore, nct, out_dtype="f16"):
    """Device outputs [ndual*? ...] per core -> full [B, NUM_WORDS, H]."""
    out = np.zeros((B, NUM_WORDS, H), np.float32)
    oscale = OUT_F8_SCALE if out_dtype == "f8e3" else 1.0
    for c in range(N_CORES):
        yc = np.asarray(res_y[c], dtype=np.float32) / oscale
        # yc shape [ndual//OG, P, OG*H] -> [ndual, P, H] word-chunk-major
        ndual = nct // 2
        og = yc.shape[2] // H
        yd = yc.reshape(ndual // og, P, og, H).transpose(0, 2, 1, 3).reshape(
            ndual, P, H
        )
        for ci, k in enumerate(percore[c]):
            if k["ntok"] == 0:
                continue
            half = ci % 2
            rows = yd[ci // 2, half * WC : half * WC + k["nw"]]
            out[k["sample"], k["w0"] : k["w0"] + k["nw"]] = rows
    return out


def _build2(
    nct,
    reps=1,
    dyn_reps=1,
    in_group=8,
    out_group=4,
    in_dtype="f8e3",
    out_dtype="f16",
    in_alt=False,
    out_engine="scalar",
    oh_engine="vector",
    ev_engine="alt",
    x_bufs=5,
    oh_bufs=6,
    ev_bufs=4,
    ps_bufs=4,
    mm_order="interleave",
    oh_mode="per_chunk",
    ps_pair=False,
    hints=False,
    do_in=True,
    do_mm=True,
    do_ev=True,
    do_out=True,
):
    from contextlib import nullcontext
    import concourse.bacc as bacc
    import concourse.tile as tile
    from concourse import mybir

    nc = bacc.Bacc(
        "TRN2",
        target_bir_lowering=False,
        debug=False,
        enable_asserts=False,
        num_devices=N_CORES,
    )
    f32 = mybir.dt.float32
    fin = {"f16": mybir.dt.float16, "f8e3": mybir.dt.float8e3, "f32": f32}[in_dtype]
    fout = {"f16": mybir.dt.float16, "f8e3": mybir.dt.float8e3, "f32": f32}[
        out_dtype
    ]
    IG, OG = in_group, out_group
    assert nct % (2 * OG) == 0 and nct % IG == 0 and IG % 2 == 0
    if ps_pair:
        assert OG == 2, "ps_pair requires out_group=2 (one ev tile per pair)"
        ps_bufs = min(ps_bufs, 2)  # [P, 2*H] f32 = 3 banks; 2 bufs = 6 of 8
    NDUAL = nct // 2
    NG = nct // IG
    x = nc.dram_tensor("x", [NG, P, IG * H], fin, kind="ExternalInput").ap()
    meta = nc.dram_tensor("meta", [P, nct + NDUAL], f32, kind="ExternalInput").ap()
    y = nc.dram_tensor(
        "y", [NDUAL // OG, P, OG * H], fout, kind="ExternalOutput"
    ).ap()

    with tile.TileContext(nc) as tc:
        with (
            tc.tile_pool(name="const", bufs=1) as const_pool,
            tc.tile_pool(name="xin", bufs=x_bufs) as x_pool,
            tc.tile_pool(name="oh", bufs=oh_bufs) as oh_pool,
            tc.tile_pool(name="ev", bufs=ev_bufs) as ev_pool,
            tc.tile_pool(name="psum", bufs=ps_bufs, space="PSUM") as psum_pool,
        ):
            iota_f = const_pool.tile([P, WC], f32)
            nc.gpsimd.iota(
                iota_f[:], pattern=[[1, WC]], base=0, channel_multiplier=0,
                allow_small_or_imprecise_dtypes=True,
            )
            meta_t = const_pool.tile([P, nct + NDUAL], f32, name="meta")
            nc.scalar.dma_start(out=meta_t[:], in_=meta[:, :])
            wid_t = meta_t[:, :nct]
            rcp_t = meta_t[:, nct:]

            oh_eng = nc.gpsimd if oh_engine == "gpsimd" else nc.vector

            def dma_eng(g):
                if in_alt == "gpsimd":
                    return nc.gpsimd if g % 2 == 1 else nc.sync
                if in_alt == "3way":
                    return (nc.sync, nc.scalar, nc.gpsimd)[g % 3]
                if in_alt:
                    return nc.scalar if g % 2 == 1 else nc.sync
                return nc.sync

            def emit(rep):
                xg_tiles = {}
                ohg_tiles = {}
                og_tile = [None]
                pair_ps = [None]
                for dd in range(NDUAL):
                    c0, c1 = 2 * dd, 2 * dd + 1
                    g = c0 // IG
                    if g not in xg_tiles:
                        xt = x_pool.tile(
                            [P, IG, H], fin, name=f"xt_{rep}_{g}", tag="xt"
                        )
                        if do_in:
                            src = x[g, :, :].rearrange("p (a h) -> p a h", a=IG)
                            dma_eng(g).dma_start(out=xt[:], in_=src)
                        xg_tiles[g] = xt
                        xg_tiles.pop(g - x_bufs + 1, None)
                    xt0 = xg_tiles[g][:, c0 % IG, :]
                    xt1 = xg_tiles[g][:, c1 % IG, :]
                    if oh_mode == "mega":
                        # one is_equal builds one-hots for all IG chunks of
                        # the group: iota broadcast against per-chunk wid
                        if g not in ohg_tiles:
                            ohg = oh_pool.tile(
                                [P, IG, WC], fin, name=f"oh_{rep}_{g}", tag="oh"
                            )
                            in0 = (
                                wid_t[:, g * IG : (g + 1) * IG]
                                .unsqueeze(2)
                                .broadcast_to([P, IG, WC])
                            )
                            in1 = iota_f[:].unsqueeze(1).broadcast_to(
                                [P, IG, WC]
                            )
                            oh_eng.tensor_tensor(
                                out=ohg[:], in0=in0, in1=in1,
                                op=mybir.AluOpType.is_equal,
                            )
                            ohg_tiles[g] = ohg
                            ohg_tiles.pop(g - oh_bufs + 1, None)
                        ohg = ohg_tiles[g]
                        oh0 = ohg[:, c0 % IG, :]
                        oh1 = ohg[:, c1 % IG, :]
                    else:
                        oh = oh_pool.tile(
                            [P, 2 * WC], fin, name=f"oh_{rep}_{dd}", tag="oh"
                        )
                        build = oh_eng.tensor_scalar
                        build(
                            out=oh[:, :WC], in0=iota_f[:],
                            scalar1=wid_t[:, c0 : c0 + 1],
                            scalar2=None, op0=mybir.AluOpType.is_equal,
                        )
                        build(
                            out=oh[:, WC:], in0=iota_f[:],
                            scalar1=wid_t[:, c1 : c1 + 1],
                            scalar2=None, op0=mybir.AluOpType.is_equal,
                        )
                        oh0 = oh[:, :WC]
                        oh1 = oh[:, WC:]
                    if do_mm:
                        if ps_pair:
                            # one [P, 2*H] psum tile (3 banks) per 2 duals;
                            # split ranges keep every matmul within a bank
                            j = dd % 2
                            if j == 0:
                                ps_t = psum_pool.tile(
                                    [P, 2 * H], f32,
                                    name=f"ps_{rep}_{dd // 2}", tag="ps",
                                )
                                pair_ps[0] = ps_t
                            ps_t = pair_ps[0]
                            splits = (
                                ((0, 512), (512, H)) if j == 0
                                else ((0, 256), (256, H))
                            )
                            base = j * H
                            mms = []
                            for lo, hi in splits:
                                mms.append((
                                    ps_t[0:WC, base + lo : base + hi], oh0,
                                    xt0[:, lo:hi], (0, 0)))
                                mms.append((
                                    ps_t[WC:P, base + lo : base + hi], oh1,
                                    xt1[:, lo:hi], (0, 64)))
                        else:
                            ps = psum_pool.tile(
                                [P, H], f32, name=f"ps_{rep}_{dd}", tag="ps"
                            )
                            mms = []
                            for lo, hi in NSPLITS:
                                mms.append((ps[0:WC, lo:hi], oh0,
                                            xt0[:, lo:hi], (0, 0)))
                                mms.append((ps[WC:P, lo:hi], oh1,
                                            xt1[:, lo:hi], (0, 64)))
                        if mm_order == "chunk":
                            mms = [mms[0], mms[2], mms[1], mms[3]]
                        for out_ap, lhsT, rhs, tp in mms:
                            nc.tensor.matmul(
                                out=out_ap, lhsT=lhsT, rhs=rhs,
                                start=True, stop=True, tile_position=tp,
                            )
                    if dd % OG == 0:
                        og_tile[0] = ev_pool.tile(
                            [P, OG, H], fout, name=f"out_{rep}_{dd}", tag="out"
                        )
                    out_sb = og_tile[0][:, dd % OG, :]
                    if do_mm and do_ev and ps_pair:
                        # evict once per pair tile: alternate one wide
                        # vector tensor_tensor (rcp broadcast over H) with
                        # two scalar muls
                        if dd % 2 == 1:
                            ps_t = pair_ps[0]
                            k = dd // 2
                            if k % 2 == 0:
                                src = ps_t[:].rearrange(
                                    "p (a h) -> p a h", a=2
                                )
                                rcpb = (
                                    rcp_t[:, dd - 1 : dd + 1]
                                    .unsqueeze(2)
                                    .broadcast_to([P, 2, H])
                                )
                                nc.vector.tensor_tensor(
                                    out=og_tile[0][:], in0=src, in1=rcpb,
                                    op=mybir.AluOpType.mult,
                                )
                            else:
                                for j in (0, 1):
                                    nc.scalar.mul(
                                        out=og_tile[0][:, j, :],
                                        in_=ps_t[:, j * H : (j + 1) * H],
                                        mul=rcp_t[:, dd - 1 + j : dd + j],
                                    )
                    elif do_mm and do_ev and ev_engine == "split":
                        # two concurrent half-evicts per dual: scalar takes
                        # the 512 half, vector the 256 half - halves the
                        # per-dual evict latency in the psum-recycle chain
                        nc.scalar.mul(
                            out=out_sb[:, 0:512], in_=ps[:, 0:512],
                            mul=rcp_t[:, dd : dd + 1],
                        )
                        nc.vector.tensor_scalar(
                            out=out_sb[:, 512:H], in0=ps[:, 512:H],
                            scalar1=rcp_t[:, dd : dd + 1],
                            scalar2=None, op0=mybir.AluOpType.mult,
                        )
                    elif do_mm and do_ev:
                        if ev_engine == "alt":
                            eng = "scalar" if dd % 2 == 0 else "vector"
                        elif ev_engine == "alt2":
                            eng = "scalar" if dd % 3 == 0 else "vector"
                        elif ev_engine == "alt3":
                            eng = "vector" if dd % 3 == 0 else "scalar"
                        else:
                            eng = ev_engine
                        if eng == "vector":
                            nc.vector.tensor_scalar(
                                out=out_sb, in0=ps[:], scalar1=rcp_t[:, dd : dd + 1],
                                scalar2=None, op0=mybir.AluOpType.mult,
                            )
                        else:
                            nc.scalar.mul(
                                out=out_sb, in_=ps[:], mul=rcp_t[:, dd : dd + 1]
                            )
                    elif do_ev:
                        nc.gpsimd.memset(out_sb, 0.0)
                    if do_out and dd % OG == OG - 1:
                        dst = y[dd // OG, :, :].rearrange("p (a h) -> p a h", a=OG)
                        if out_engine == "alt":
                            oe = nc.scalar if (dd // OG) % 2 == 0 else nc.sync
                        elif out_engine == "gpsimd":
                            oe = nc.gpsimd
                        else:
                            oe = nc.scalar
                        oe.dma_start(out=dst, in_=og_tile[0][:])

            fkw = {}
            if hints and dyn_reps > 1:
                fkw = dict(
                    back_edge_label="bh", hint_engines=list(mybir.ALL_ENGINES)
                )
            loop_cm = (
                tc.For_i(0, dyn_reps, 1, **fkw) if dyn_reps > 1 else nullcontext()
            )
            with loop_cm:
                if hints and dyn_reps > 1:
                    tc.mark_branch_hint_location(
                        "bh", engines=list(mybir.ALL_ENGINES)
                    )
                for rep in range(reps):
                    emit(rep)

    nc.compile()
    return nc


BEST2 = {
    "in_group": 4,
    "out_group": 2,
    "in_dtype": "f8e3",
    "out_dtype": "f8e3",
    "in_alt": False,
    "oh_engine": "vector",
    "ev_engine": "alt",
    "x_bufs": 8,
    "oh_bufs": 10,
    "ev_bufs": 6,
}
R_UNROLL = 32  # bodies per hardware-loop iteration for steady-state timing


def _prep_kwargs2(bkw):
    return {
        "in_dtype": bkw.get("in_dtype", "f8e3"),
        "in_group": bkw.get("in_group", 8),
        "out_dtype": bkw.get("out_dtype", "f16"),
    }


def kernel_v2(hidden_states, word_ids):
    import concourse.bass_utils as bass_utils

    wid = np.asarray(word_ids, dtype=np.int32)
    percore, nct = _plan2(wid, pad_mult=_pad_mult2(BEST2))
    nc = _build2(nct, **BEST2)
    in_maps = _prep_inputs2(hidden_states, word_ids, percore, nct,
                            **_prep_kwargs2(BEST2))
    res = bass_utils.run_bass_kernel_spmd(nc, in_maps, core_ids=list(range(N_CORES)))
    ys = [res.results[c]["y"] for c in range(N_CORES)]
    return _unshard2(ys, percore, nct, out_dtype=BEST2.get("out_dtype", "f16"))


def _pad_mult2(bkw):
    ig = bkw.get("in_group", 8)
    og = bkw.get("out_group", 4)
    import math

    return math.lcm(ig, 2 * og)


def _prep_inputs(hidden_states, word_ids, perm=None, in_dtype="f16",
                 in_layout="rowmajor", in_group=4, rcp_mode="onehot"):
    hs = np.asarray(hidden_states, dtype=np.float32)
    if in_dtype == "f8e3":
        import ml_dtypes

        hs = np.ascontiguousarray((hs * FP8_SCALE).astype(ml_dtypes.float8_e3m4))
        descale = 1.0 / FP8_SCALE
    else:
        np_in = np.float16 if in_dtype == "f16" else np.float32
        hs = np.ascontiguousarray(hs.astype(np_in))
        descale = 1.0
    wid = np.minimum(np.asarray(word_ids, dtype=np.int32), NUM_WORDS - 1)
    assert hs.shape == (B, S, H) and wid.shape == (B, S)
    if perm is None:
        perm = np.arange(B, dtype=np.int64).reshape(N_CORES, SPC)
    # [B, S] -> [B, P, KT]: element (p, t) = token t*P + p
    widf = np.ascontiguousarray(
        wid.astype(np.float32).reshape(B, KT, P).transpose(0, 2, 1)
    )
    if rcp_mode == "evict":
        # per-word 1/count in [B, P, NW]: element (p, j) = word j*P + p
        rt = np.zeros((B, NUM_WORDS), np.float32)
        for b in range(B):
            w = wid[b]
            counts = np.bincount(w[w >= 0], minlength=NUM_WORDS)
            nz = counts > 0
            rt[b, nz] = descale / counts[nz]
        rt = np.ascontiguousarray(rt.reshape(B, NW, P).transpose(0, 2, 1))
    else:
        r = _recip_counts(wid)
        rt = np.ascontiguousarray(r.reshape(B, KT, P).transpose(0, 2, 1))
    in_maps = []
    for c in range(N_CORES):
        sl = list(perm[c])
        if in_layout == "pmajor":
            IG = in_group
            xc = np.ascontiguousarray(
                hs[sl]
                .reshape(SPC, KT // IG, IG, P, H)
                .transpose(0, 1, 3, 2, 4)
                .reshape(SPC * KT // IG, P, IG * H)
            )
        elif in_layout == "pmajor_fused":
            IG = in_group
            xc = np.ascontiguousarray(
                hs[sl]
                .reshape(SPC, KT // IG, IG, P, H)
                .transpose(1, 3, 0, 2, 4)  # [KT//IG, P, SPC, IG, H]
                .reshape(KT // IG, P, SPC * IG * H)
            )
        else:
            xc = np.ascontiguousarray(hs[sl].reshape(SPC * S, H))
        mc = np.ascontiguousarray(
            np.concatenate(
                [np.concatenate([widf[s], rt[s]], axis=1) for s in sl], axis=1
            )
        )
        in_maps.append({"x": xc, "meta": mc})
    return in_maps


# Best-known configuration (applied by kernel(); bench.py overrides).
# fp8-e3m4 input (host pre-scaled by 2, descaled via the per-word 1/count
# applied at PSUM eviction, which keeps the one-hot exactly representable):
# end-to-end rel err 1.33e-2 vs the 2e-2 gate; halves input HBM traffic.
BEST = {
    "in_layout": "pmajor_fused",
    "in_group": 4,
    "out_layout": "pmajor",
    "out_group": 4,
    "rcp_mode": "evict",
    "in_dtype": "f8e3",
}


def _prep_kwargs(bkw):
    return {
        "in_dtype": bkw.get("in_dtype", "f16"),
        "in_layout": bkw.get("in_layout", "rowmajor"),
        "in_group": bkw.get("in_group", 4),
        "rcp_mode": bkw.get("rcp_mode", "onehot"),
    }


def _unshard(yc, bkw):
    """Per-core device output -> [SPC, NUM_WORDS, H]."""
    if bkw.get("out_layout", "rowmajor") == "pmajor":
        og = bkw.get("out_group", 2)
        return (
            yc.reshape(SPC, NW // og, P, og, H)
            .transpose(0, 1, 3, 2, 4)
            .reshape(SPC, NUM_WORDS, H)
        )
    return yc.reshape(SPC, NUM_WORDS, H)


def kernel_v1(hidden_states, word_ids):
    import concourse.bass_utils as bass_utils

    wid = np.asarray(word_ids, dtype=np.int32)
    plans, perm = _plan(wid)
    nc = _build(plans, **BEST)
    in_maps = _prep_inputs(hidden_states, word_ids, perm, **_prep_kwargs(BEST))
    res = bass_utils.run_bass_kernel_spmd(nc, in_maps, core_ids=list(range(N_CORES)))
    out = np.empty((B, NUM_WORDS, H), np.float32)
    for c in range(N_CORES):
        yc = np.asarray(res.results[c]["y"], dtype=np.float32)
        yb = _unshard(yc, BEST)
        for slot in range(SPC):
            out[perm[c][slot]] = yb[slot]
    return out


def kernel(hidden_states, word_ids):
    return kernel_v2(hidden_states, word_ids)

